# revision 2
# baseline (speedup 1.0000x reference)
"""Trainium2 Bass kernel v2: sampled logistic-regression forward.

Math per data row r, sample s:
    mean_r = X[r] . w_mu
    var_r  = sum_d X[r,d]^2 * exp(w_log_var[d])
    out[r,s] = sigmoid( sqrt(var_r) * z[s] + mean_r )

Full shapes: X [500000, 64] f32, w_mu/w_log_var [64], z [128].
Output [500000, 128] f32.  Harness gate: rel err < 2e-2 -> fp16 I/O is safe
(measured end-to-end max abs err ~5e-3 on the real seed-0 data).

Dataflow (per core, data-parallel rows/8):
  Host uploads X transposed + parity-packed fp16: xd [128, C]; partition
  d<64 holds X^T[d, even rows], d>=64 holds X^T[d, odd rows]; column
  n <-> row pair (2n, 2n+1). 4KB DMA lines (the old kernel used 256B
  lines at half DMA rate - its bottleneck).

  Per batch of 16 groups (4 slots x 4 groups x 512 columns = 16384 rows):
  - DVE: xsq = xd*xd (fp16 2x mode).
  - PE stats into ONE [128, 512] psum tile, row = cls*32 + slot*8 + gp
    (gp = par*4 + q; cls in {mean, mean-dup, var, var-dup}):
    per (slot, group) an M=64 matmul pair - x-pass with the wmu pattern
    into rows 0:64, xsq-pass with the exp(lv) pattern into rows 64:128 -
    accumulating across slots/groups via zeroed lhsT columns.  The
    mean/var reductions thus run on PE (DVE tensor_reduce is 1x-rate and
    would bottleneck at ~115us).
  - Splits, all 32-aligned full-width ops: stb = fp16(pst) (mh); ml =
    mean - mh into rows 32:64 (keeps mean at ~fp24); Newton rsqrt
    (bit-trick seed on DVE, 2 NR iterations on the otherwise-idle GPSIMD);
    sh = fp16(std) into rows 64:128.
  - PE affine per (slot, j, par): K=128 block-diagonal matmul
      out[(q,ss), n] = sum_k zpat[k, (q,ss)] * stb[k, n]
    with class coefficients {1, 1, zh, zl}: act = mh+ml + sh*(zh+zl).
    The z hi/lo split removes the fp16-z rounding term.
  - ACT: Sigmoid (single table) PSUM->SBUF fp16.
  - DMA out fp16 to [2*NS, C] (1KB lines); host transposes back.

Engine budget per core (cost model): DMA ~68us (bound), ACT ~62us,
PE ~59us, DVE ~33us, GPSIMD ~26us.
"""

from contextlib import ExitStack

import numpy as np

import concourse.bacc as bacc
import concourse.tile as tile
from concourse import mybir
from concourse.bass_utils import run_bass_kernel_spmd

N_CORES = 8
D = 64
NS = 128
GRP = 512          # columns (row-pairs) per group
GPS = 4            # groups per slot (one in-tile)
SPB = 4            # slots per batch
TCOLS = GPS * GRP  # in-tile columns (4KB DMA lines)

RSQRT_MAGIC = 0x5F3759DF
F32 = mybir.dt.float32
F16 = mybir.dt.float16
U32 = mybir.dt.uint32


def build_program(cols: int):
    """cols = padded row-pairs per core (multiple of GRP)."""
    assert cols % GRP == 0
    ngroups = cols // GRP
    nslots = (ngroups + GPS - 1) // GPS

    nc = bacc.Bacc(
        "TRN2",
        target_bir_lowering=False,
        debug=False,
        num_devices=N_CORES,
    )

    xd = nc.dram_tensor("xd", [128, cols], F16, kind="ExternalInput")
    # lhsT variants per (slot, group): wx/wq [128, 16*64]
    wx = nc.dram_tensor("wx", [128, SPB * GPS * 64], F16, kind="ExternalInput")
    wq = nc.dram_tensor("wq", [128, SPB * GPS * 64], F16, kind="ExternalInput")
    # zpat per (slot, j, par): [128, 4*8*NS]
    zp = nc.dram_tensor("zp", [128, SPB * 8 * NS], F16, kind="ExternalInput")
    # raw per-slot dump: row = slot*128 + (q,ss); col = j*1024 + par*512 + n
    out_d = nc.dram_tensor("out", [nslots * 128, 4 * 2 * GRP], F16,
                           kind="ExternalOutput")

    with tile.TileContext(nc) as tc, ExitStack() as ctx:
        singles = ctx.enter_context(tc.tile_pool(name="singles", bufs=1))
        xin = ctx.enter_context(tc.tile_pool(name="xin", bufs=4))
        sqp = ctx.enter_context(tc.tile_pool(name="sqp", bufs=4))
        statp = ctx.enter_context(tc.tile_pool(name="statp", bufs=2))
        varp = ctx.enter_context(tc.tile_pool(name="varp", bufs=2))
        nrp = ctx.enter_context(tc.tile_pool(name="nrp", bufs=2))
        outp = ctx.enter_context(tc.tile_pool(name="outp", bufs=4))
        pst_pool = ctx.enter_context(tc.tile_pool(name="pst", bufs=2, space="PSUM"))
        paff_pool = ctx.enter_context(tc.tile_pool(name="paff", bufs=3, space="PSUM"))

        pre = []
        wx_sb = singles.tile([128, SPB * GPS * 64], F16)
        nc.sync.dma_start(out=wx_sb, in_=wx[:, :])
        wq_sb = singles.tile([128, SPB * GPS * 64], F16)
        nc.sync.dma_start(out=wq_sb, in_=wq[:, :])
        zp_sb = singles.tile([128, SPB * 8 * NS], F16)
        nc.sync.dma_start(out=zp_sb, in_=zp[:, :])
        magic_sb = singles.tile([128, GRP], U32)
        nc.vector.memset(magic_sb, RSQRT_MAGIC)
        one_sb = singles.tile([128, 1], U32)
        nc.vector.memset(one_sb, 1)

        sizes = []
        while sum(sizes) < nslots:
            sizes.append(min(SPB, nslots - sum(sizes)))

        def emit_stats_and_splits(b0, sb_n):
            pst = pst_pool.tile([128, GRP], F32)
            ngs = []
            # (t, q) pairs in this batch, to place start/stop flags
            pairs = []
            for t in range(sb_n):
                S = b0 + t
                ng = min(GPS, ngroups - S * GPS)
                ngs.append(ng)
                pairs += [(t, q) for q in range(ng)]
            npair = len(pairs)

            pi = 0
            for t in range(sb_n):
                S = b0 + t
                ng = ngs[t]
                w = ng * GRP
                if S < len(pre):
                    xt = pre[S]
                else:
                    xt = xin.tile([128, TCOLS], F16)
                    nc.sync.dma_start(
                        out=xt[:, :w], in_=xd[:, S * TCOLS : S * TCOLS + w]
                    )
                sq = sqp.tile([128, TCOLS], F16)
                nc.vector.tensor_mul(sq[:, :w], xt[:, :w], xt[:, :w])
                for q in range(ng):
                    s = slice(q * GRP, (q + 1) * GRP)
                    v = (t * GPS + q) * 64
                    nc.tensor.matmul(
                        pst[0:64, :],
                        lhsT=wx_sb[:, v : v + 64],
                        rhs=xt[:, s],
                        start=(pi == 0),
                        stop=(pi == npair - 1),
                    )
                    nc.tensor.matmul(
                        pst[64:128, :],
                        lhsT=wq_sb[:, v : v + 64],
                        rhs=sq[:, s],
                        start=(pi == 0),
                        stop=(pi == npair - 1),
                    )
                    pi += 1

            # ---- split cascade (all ops 32-aligned, full width) ----
            stb = statp.tile([128, GRP], F16)
            vs = varp.tile([128, GRP], F32)
            yb = nrp.tile([128, GRP], U32)
            tn = nrp.tile([128, GRP], F32)

            # fp16 snapshot: rows 0:64 -> mh (x2), rows 64:128 junk for now
            nc.vector.tensor_copy(stb, pst)
            # f32 copy for newton (mean rows junk there)
            nc.vector.tensor_copy(vs, pst)
            # ml = mean - mh into the dup-mean class rows 32:64
            nc.vector.tensor_sub(stb[32:64, :], vs[32:64, :], stb[32:64, :])
            # newton-rsqrt seed on DVE: y = magic - (var >> 1)
            nc.vector.tensor_scalar(
                yb, vs.bitcast(U32), one_sb[:, 0:1], None,
                op0=mybir.AluOpType.logical_shift_right,
            )
            nc.vector.scalar_tensor_tensor(
                out=yb,
                in0=magic_sb,
                scalar=0,
                in1=yb,
                op0=mybir.AluOpType.bypass,
                op1=mybir.AluOpType.subtract,
            )
            y = yb.bitcast(F32)
            # 2 NR iterations + std = var*y on GPSIMD (mean rows junk, unread)
            # NR order (y*var)*y keeps zero-var rows finite (y ~ 1.3e19
            # there; y*y would overflow f32 on the second iteration and
            # the resulting NaN would poison the whole affine contraction)
            for _ in range(2):
                nc.gpsimd.tensor_mul(tn, y, vs)
                nc.gpsimd.tensor_mul(tn, tn, y)
                nc.gpsimd.tensor_scalar(
                    tn, tn, -0.5, 1.5,
                    op0=mybir.AluOpType.mult,
                    op1=mybir.AluOpType.add,
                )
                nc.gpsimd.tensor_mul(y, y, tn)
            nc.gpsimd.tensor_mul(vs[64:128, :], vs[64:128, :], y[64:128, :])
            # sh into both var classes (rows 64:128), fp16
            nc.vector.tensor_copy(stb[64:128, :], vs[64:128, :])
            return stb

        def emit_affine(b0, sb_n, stb):
            for t in range(sb_n):
                S = b0 + t
                outb = outp.tile([128, 8 * GRP], F16)
                for j in range(4):  # sample block of 32
                    pa = paff_pool.tile([128, 2 * GRP], F32)
                    for par in range(2):
                        zc = (t * 8 + j * 2 + par) * NS
                        nc.tensor.matmul(
                            pa[:, par * GRP : (par + 1) * GRP],
                            lhsT=zp_sb[:, zc : zc + NS],
                            rhs=stb,
                            start=True,
                            stop=True,
                        )
                    nc.scalar.activation(
                        out=outb[:, j * 2 * GRP : (j + 1) * 2 * GRP],
                        in_=pa,
                        func=mybir.ActivationFunctionType.Sigmoid,
                    )
                # one raw-layout store per slot: 128 descriptors of 8KB
                # (descriptor generation costs ~6ns/desc on the issuing
                # engine's sequencer - 1KB lines would eat ~90us of it)
                nc.sync.dma_start(
                    out=out_d[S * 128 : (S + 1) * 128, :], in_=outb
                )

        b0 = 0
        for sb_n in sizes:
            stb = emit_stats_and_splits(b0, sb_n)
            emit_affine(b0, sb_n, stb)
            b0 += sb_n

    nc.finalize()
    return nc


def _host_inputs(X: np.ndarray, w_mu, w_log_var, z, cols: int):
    n = X.shape[0]
    rows = n // N_CORES
    f16 = np.float16
    wmu = np.asarray(w_mu, dtype=np.float64)
    elv = np.exp(np.asarray(w_log_var, dtype=np.float64))
    zz = np.asarray(z, dtype=np.float64)

    # psum row map: row = cls*32 + t*8 + par*4 + q
    # wx (x-pass lhsT, M=64 = cls 0|1) / wq (xsq-pass, M=64 = cls 2|3):
    # variant per (t, q) at cols (t*GPS+q)*64.
    wx = np.zeros((128, SPB * GPS * 64), dtype=f16)
    wq = np.zeros((128, SPB * GPS * 64), dtype=f16)
    for t in range(SPB):
        for q in range(GPS):
            v = (t * GPS + q) * 64
            for cls in range(2):
                for par in range(2):
                    m = v + cls * 32 + t * 8 + par * 4 + q
                    rsl = slice(0, D) if par == 0 else slice(D, 128)
                    wx[rsl, m] = wmu.astype(f16)
                    wq[rsl, m] = elv.astype(f16)

    zh = zz.astype(f16)
    zl = (zz - zh.astype(np.float64)).astype(f16)
    # zpat [128, (t, j, par), NS]: row k = cls*32 + t'*8 + par'*4 + q;
    # col m = (q, ss): coef for sample s = j*32+ss, nonzero iff
    # t'==t and par'==par:  cls0 -> 1, cls1 -> 1, cls2 -> zh, cls3 -> zl.
    comp = np.stack([np.ones(NS), np.ones(NS), zh.astype(np.float64),
                     zl.astype(np.float64)])  # [4, NS]
    zpat = np.zeros((128, SPB, 4, 2, NS), dtype=f16)
    for k in range(128):
        cls, tp, parp, q = k // 32, (k % 32) // 8, (k % 8) // 4, k % 4
        for j in range(4):
            m0 = q * 32
            zpat[k, tp, j, parp, m0 : m0 + 32] = comp[
                cls, j * 32 : (j + 1) * 32
            ].astype(f16)
    zp_arr = np.ascontiguousarray(zpat.reshape(128, SPB * 8 * NS))

    in_maps = []
    for i in range(N_CORES):
        Xi = np.asarray(X[i * rows : (i + 1) * rows], dtype=f16)
        Xp = np.zeros((2 * cols, D), dtype=f16)
        Xp[:rows] = Xi
        XT = Xp.T  # [64, 2*cols] view
        xd = np.empty((128, cols), dtype=f16)
        xd[0:D] = XT[:, 0::2]
        xd[D:128] = XT[:, 1::2]
        in_maps.append({"xd": xd, "wx": wx, "wq": wq, "zp": zp_arr})
    return in_maps


_PROGRAM_CACHE: dict[int, object] = {}


def run(X, w_mu, w_log_var, z, trace=False):
    X = np.asarray(X)
    n = X.shape[0]
    assert n % (2 * N_CORES) == 0
    rows = n // N_CORES
    cols = ((rows + 1) // 2 + GRP - 1) // GRP * GRP
    if cols not in _PROGRAM_CACHE:
        _PROGRAM_CACHE[cols] = build_program(cols)
    nc = _PROGRAM_CACHE[cols]

    in_maps = _host_inputs(X, w_mu, w_log_var, z, cols)
    res = run_bass_kernel_spmd(nc, in_maps, list(range(N_CORES)), trace=trace)
    nslots = (cols // GRP + GPS - 1) // GPS
    outs = []
    for i in range(N_CORES):
        o = res.results[i]["out"]  # [nslots*128, 4096] f16
        arr = o.reshape(nslots, 4, 32, 4, 2, GRP)  # [S, q, ss, j, par, n]
        full = (
            arr.transpose(0, 1, 5, 4, 3, 2)  # [S, q, n, par, j, ss]
            .reshape(2 * nslots * GPS * GRP, NS)
            .astype(np.float32)
        )
        outs.append(full[:rows])
    return np.concatenate(outs, axis=0), res


def kernel(X, w_mu, w_log_var, z):
    full, _ = run(X, w_mu, w_log_var, z, trace=False)
    return full


# revision 3
# speedup vs baseline: 1.0331x; 1.0331x over previous
"""Trainium2 Bass kernel v2: sampled logistic-regression forward.

Math per data row r, sample s:
    mean_r = X[r] . w_mu
    var_r  = sum_d X[r,d]^2 * exp(w_log_var[d])
    out[r,s] = sigmoid( sqrt(var_r) * z[s] + mean_r )

Full shapes: X [500000, 64] f32, w_mu/w_log_var [64], z [128].
Output [500000, 128] f32.  Harness gate: rel err < 2e-2 -> fp16 I/O is safe
(measured end-to-end max abs err ~5e-3 on the real seed-0 data).

Dataflow (per core, data-parallel rows/8):
  Host uploads X transposed + parity-packed fp16: xd [128, C]; partition
  d<64 holds X^T[d, even rows], d>=64 holds X^T[d, odd rows]; column
  n <-> row pair (2n, 2n+1). 4KB DMA lines (the old kernel used 256B
  lines at half DMA rate - its bottleneck).

  Per batch of 16 groups (4 slots x 4 groups x 512 columns = 16384 rows):
  - DVE: xsq = xd*xd (fp16 2x mode).
  - PE stats into ONE [128, 512] psum tile, row = cls*32 + slot*8 + gp
    (gp = par*4 + q; cls in {mean, mean-dup, var, var-dup}):
    per (slot, group) an M=64 matmul pair - x-pass with the wmu pattern
    into rows 0:64, xsq-pass with the exp(lv) pattern into rows 64:128 -
    accumulating across slots/groups via zeroed lhsT columns.  The
    mean/var reductions thus run on PE (DVE tensor_reduce is 1x-rate and
    would bottleneck at ~115us).
  - Splits, all 32-aligned full-width ops: stb = fp16(pst) (mh); ml =
    mean - mh into rows 32:64 (keeps mean at ~fp24); Newton rsqrt
    (bit-trick seed on DVE, 2 NR iterations on the otherwise-idle GPSIMD);
    sh = fp16(std) into rows 64:128.
  - PE affine per (slot, j, par): K=128 block-diagonal matmul
      out[(q,ss), n] = sum_k zpat[k, (q,ss)] * stb[k, n]
    with class coefficients {1, 1, zh, zl}: act = mh+ml + sh*(zh+zl).
    The z hi/lo split removes the fp16-z rounding term.
  - ACT: Sigmoid (single table) PSUM->SBUF fp16.
  - DMA out fp16 to [2*NS, C] (1KB lines); host transposes back.

Engine budget per core (cost model): DMA ~68us (bound), ACT ~62us,
PE ~59us, DVE ~33us, GPSIMD ~26us.
"""

from contextlib import ExitStack

import numpy as np

import concourse.bacc as bacc
import concourse.tile as tile
from concourse import mybir
from concourse.bass_utils import run_bass_kernel_spmd

N_CORES = 8
D = 64
NS = 128
GRP = 512          # columns (row-pairs) per group
GPS = 4            # groups per slot (one in-tile)
SPB = 4            # slots per batch
TCOLS = GPS * GRP  # in-tile columns (4KB DMA lines)

RSQRT_MAGIC = 0x5F3759DF
F32 = mybir.dt.float32
F16 = mybir.dt.float16
U32 = mybir.dt.uint32


def build_program(cols: int):
    """cols = padded row-pairs per core (multiple of GRP)."""
    assert cols % GRP == 0
    ngroups = cols // GRP
    nslots = (ngroups + GPS - 1) // GPS

    nc = bacc.Bacc(
        "TRN2",
        target_bir_lowering=False,
        debug=False,
        num_devices=N_CORES,
    )

    xd = nc.dram_tensor("xd", [128, cols], F16, kind="ExternalInput")
    # lhsT variants per (slot, group): wx/wq [128, 16*64]
    wx = nc.dram_tensor("wx", [128, SPB * GPS * 64], F16, kind="ExternalInput")
    wq = nc.dram_tensor("wq", [128, SPB * GPS * 64], F16, kind="ExternalInput")
    # zpat per (slot, j, par): [128, 4*8*NS]
    zp = nc.dram_tensor("zp", [128, SPB * 8 * NS], F16, kind="ExternalInput")
    # raw per-slot dump: row = slot*128 + (q,ss); col = j*1024 + par*512 + n
    out_d = nc.dram_tensor("out", [nslots * 128, 4 * 2 * GRP], F16,
                           kind="ExternalOutput")

    with tile.TileContext(nc) as tc, ExitStack() as ctx:
        singles = ctx.enter_context(tc.tile_pool(name="singles", bufs=1))
        xin = ctx.enter_context(tc.tile_pool(name="xin", bufs=4))
        sqp = ctx.enter_context(tc.tile_pool(name="sqp", bufs=4))
        statp = ctx.enter_context(tc.tile_pool(name="statp", bufs=2))
        varp = ctx.enter_context(tc.tile_pool(name="varp", bufs=2))
        nrp = ctx.enter_context(tc.tile_pool(name="nrp", bufs=2))
        outp = ctx.enter_context(tc.tile_pool(name="outp", bufs=4))
        pst_pool = ctx.enter_context(tc.tile_pool(name="pst", bufs=2, space="PSUM"))
        paff_pool = ctx.enter_context(tc.tile_pool(name="paff", bufs=3, space="PSUM"))

        pre = []
        wx_sb = singles.tile([128, SPB * GPS * 64], F16)
        nc.sync.dma_start(out=wx_sb, in_=wx[:, :])
        wq_sb = singles.tile([128, SPB * GPS * 64], F16)
        nc.sync.dma_start(out=wq_sb, in_=wq[:, :])
        zp_sb = singles.tile([128, SPB * 8 * NS], F16)
        nc.sync.dma_start(out=zp_sb, in_=zp[:, :])
        magic_sb = singles.tile([128, GRP], U32)
        nc.vector.memset(magic_sb, RSQRT_MAGIC)
        one_sb = singles.tile([128, 1], U32)
        nc.vector.memset(one_sb, 1)

        sizes = []
        while sum(sizes) < nslots:
            sizes.append(min(SPB, nslots - sum(sizes)))

        def emit_stats_and_splits(b0, sb_n):
            pst = pst_pool.tile([128, GRP], F32)
            if b0 == 0:
                # PE p-state warm-up: junk matmuls on the consts bridge the
                # gap until the first x-tile lands, so batch 0's stats chain
                # runs at 2.4GHz instead of the cold/mid clock.  The real
                # chain's start=True overwrites the junk.
                for _ in range(24):
                    nc.tensor.matmul(
                        pst[0:64, :],
                        lhsT=wx_sb[:, 0:64],
                        rhs=wx_sb[:, 0:GRP],
                        start=True,
                        stop=True,
                    )
            ngs = []
            # (t, q) pairs in this batch, to place start/stop flags
            pairs = []
            for t in range(sb_n):
                S = b0 + t
                ng = min(GPS, ngroups - S * GPS)
                ngs.append(ng)
                pairs += [(t, q) for q in range(ng)]
            npair = len(pairs)

            pi = 0
            for t in range(sb_n):
                S = b0 + t
                ng = ngs[t]
                w = ng * GRP
                if S < len(pre):
                    xt = pre[S]
                else:
                    xt = xin.tile([128, TCOLS], F16)
                    nc.sync.dma_start(
                        out=xt[:, :w], in_=xd[:, S * TCOLS : S * TCOLS + w]
                    )
                sq = sqp.tile([128, TCOLS], F16)
                nc.vector.tensor_mul(sq[:, :w], xt[:, :w], xt[:, :w])
                for q in range(ng):
                    s = slice(q * GRP, (q + 1) * GRP)
                    v = (t * GPS + q) * 64
                    nc.tensor.matmul(
                        pst[0:64, :],
                        lhsT=wx_sb[:, v : v + 64],
                        rhs=xt[:, s],
                        start=(pi == 0),
                        stop=(pi == npair - 1),
                    )
                    nc.tensor.matmul(
                        pst[64:128, :],
                        lhsT=wq_sb[:, v : v + 64],
                        rhs=sq[:, s],
                        start=(pi == 0),
                        stop=(pi == npair - 1),
                    )
                    pi += 1

            # ---- split cascade (all ops 32-aligned, full width) ----
            stb = statp.tile([128, GRP], F16)
            vs = varp.tile([128, GRP], F32)
            yb = nrp.tile([128, GRP], U32)
            tn = nrp.tile([128, GRP], F32)

            # newton-rsqrt seed straight from PSUM, FIRST, so the GPSIMD
            # chain starts ~2us earlier (the copies then overlap with NR)
            nc.vector.tensor_scalar(
                yb, pst.bitcast(U32), one_sb[:, 0:1], None,
                op0=mybir.AluOpType.logical_shift_right,
            )
            nc.vector.scalar_tensor_tensor(
                out=yb,
                in0=magic_sb,
                scalar=0,
                in1=yb,
                op0=mybir.AluOpType.bypass,
                op1=mybir.AluOpType.subtract,
            )
            y = yb.bitcast(F32)
            # f32 copy for newton (mean rows junk there)
            nc.vector.tensor_copy(vs, pst)
            # fp16 snapshot: rows 0:64 -> mh (x2), rows 64:128 junk for now
            nc.vector.tensor_copy(stb, pst)
            # ml = mean - mh into the dup-mean class rows 32:64
            nc.vector.tensor_sub(stb[32:64, :], vs[32:64, :], stb[32:64, :])
            # 2 NR iterations + std = var*y on GPSIMD (mean rows junk, unread)
            # NR order (y*var)*y keeps zero-var rows finite (y ~ 1.3e19
            # there; y*y would overflow f32 on the second iteration and
            # the resulting NaN would poison the whole affine contraction)
            for _ in range(2):
                nc.gpsimd.tensor_mul(tn, y, vs)
                nc.gpsimd.tensor_mul(tn, tn, y)
                nc.gpsimd.tensor_scalar(
                    tn, tn, -0.5, 1.5,
                    op0=mybir.AluOpType.mult,
                    op1=mybir.AluOpType.add,
                )
                nc.gpsimd.tensor_mul(y, y, tn)
            nc.gpsimd.tensor_mul(vs[64:128, :], vs[64:128, :], y[64:128, :])
            # sh into both var classes (rows 64:128), fp16
            nc.vector.tensor_copy(stb[64:128, :], vs[64:128, :])
            return stb

        def emit_affine(b0, sb_n, stb):
            for t in range(sb_n):
                S = b0 + t
                outb = outp.tile([128, 8 * GRP], F16)
                for j in range(4):  # sample block of 32
                    pa = paff_pool.tile([128, 2 * GRP], F32)
                    for par in range(2):
                        zc = (t * 8 + j * 2 + par) * NS
                        nc.tensor.matmul(
                            pa[:, par * GRP : (par + 1) * GRP],
                            lhsT=zp_sb[:, zc : zc + NS],
                            rhs=stb,
                            start=True,
                            stop=True,
                        )
                    nc.scalar.activation(
                        out=outb[:, j * 2 * GRP : (j + 1) * 2 * GRP],
                        in_=pa,
                        func=mybir.ActivationFunctionType.Sigmoid,
                    )
                # one raw-layout store per slot: 128 descriptors of 8KB
                # (descriptor generation costs ~6ns/desc on the issuing
                # engine's sequencer - 1KB lines would eat ~90us of it)
                nc.sync.dma_start(
                    out=out_d[S * 128 : (S + 1) * 128, :], in_=outb
                )

        b0 = 0
        for sb_n in sizes:
            stb = emit_stats_and_splits(b0, sb_n)
            emit_affine(b0, sb_n, stb)
            b0 += sb_n

    nc.finalize()
    return nc


def _host_inputs(X: np.ndarray, w_mu, w_log_var, z, cols: int):
    n = X.shape[0]
    rows = n // N_CORES
    f16 = np.float16
    wmu = np.asarray(w_mu, dtype=np.float64)
    elv = np.exp(np.asarray(w_log_var, dtype=np.float64))
    zz = np.asarray(z, dtype=np.float64)

    # psum row map: row = cls*32 + t*8 + par*4 + q
    # wx (x-pass lhsT, M=64 = cls 0|1) / wq (xsq-pass, M=64 = cls 2|3):
    # variant per (t, q) at cols (t*GPS+q)*64.
    wx = np.zeros((128, SPB * GPS * 64), dtype=f16)
    wq = np.zeros((128, SPB * GPS * 64), dtype=f16)
    for t in range(SPB):
        for q in range(GPS):
            v = (t * GPS + q) * 64
            for cls in range(2):
                for par in range(2):
                    m = v + cls * 32 + t * 8 + par * 4 + q
                    rsl = slice(0, D) if par == 0 else slice(D, 128)
                    wx[rsl, m] = wmu.astype(f16)
                    wq[rsl, m] = elv.astype(f16)

    zh = zz.astype(f16)
    zl = (zz - zh.astype(np.float64)).astype(f16)
    # zpat [128, (t, j, par), NS]: row k = cls*32 + t'*8 + par'*4 + q;
    # col m = (q, ss): coef for sample s = j*32+ss, nonzero iff
    # t'==t and par'==par:  cls0 -> 1, cls1 -> 1, cls2 -> zh, cls3 -> zl.
    comp = np.stack([np.ones(NS), np.ones(NS), zh.astype(np.float64),
                     zl.astype(np.float64)])  # [4, NS]
    zpat = np.zeros((128, SPB, 4, 2, NS), dtype=f16)
    for k in range(128):
        cls, tp, parp, q = k // 32, (k % 32) // 8, (k % 8) // 4, k % 4
        for j in range(4):
            m0 = q * 32
            zpat[k, tp, j, parp, m0 : m0 + 32] = comp[
                cls, j * 32 : (j + 1) * 32
            ].astype(f16)
    zp_arr = np.ascontiguousarray(zpat.reshape(128, SPB * 8 * NS))

    in_maps = []
    for i in range(N_CORES):
        Xi = np.asarray(X[i * rows : (i + 1) * rows], dtype=f16)
        Xp = np.zeros((2 * cols, D), dtype=f16)
        Xp[:rows] = Xi
        XT = Xp.T  # [64, 2*cols] view
        xd = np.empty((128, cols), dtype=f16)
        xd[0:D] = XT[:, 0::2]
        xd[D:128] = XT[:, 1::2]
        in_maps.append({"xd": xd, "wx": wx, "wq": wq, "zp": zp_arr})
    return in_maps


_PROGRAM_CACHE: dict[int, object] = {}


def run(X, w_mu, w_log_var, z, trace=False):
    X = np.asarray(X)
    n = X.shape[0]
    assert n % (2 * N_CORES) == 0
    rows = n // N_CORES
    cols = ((rows + 1) // 2 + GRP - 1) // GRP * GRP
    if cols not in _PROGRAM_CACHE:
        _PROGRAM_CACHE[cols] = build_program(cols)
    nc = _PROGRAM_CACHE[cols]

    in_maps = _host_inputs(X, w_mu, w_log_var, z, cols)
    res = run_bass_kernel_spmd(nc, in_maps, list(range(N_CORES)), trace=trace)
    nslots = (cols // GRP + GPS - 1) // GPS
    outs = []
    for i in range(N_CORES):
        o = res.results[i]["out"]  # [nslots*128, 4096] f16
        arr = o.reshape(nslots, 4, 32, 4, 2, GRP)  # [S, q, ss, j, par, n]
        full = (
            arr.transpose(0, 1, 5, 4, 3, 2)  # [S, q, n, par, j, ss]
            .reshape(2 * nslots * GPS * GRP, NS)
            .astype(np.float32)
        )
        outs.append(full[:rows])
    return np.concatenate(outs, axis=0), res


def kernel(X, w_mu, w_log_var, z):
    full, _ = run(X, w_mu, w_log_var, z, trace=False)
    return full
